# revision 35
# baseline (speedup 1.0000x reference)
"""Causal multi-head attention (B=2, S=2048, D=1024, H=16, Dh=64) on 8 trn2 cores.

Sharding: (batch, head-group) tensor parallel. Core c handles batch c//4 and
heads [4*(c%4), 4*(c%4)+4). Each core computes its 4 heads end-to-end
(QKV projections, causal softmax attention, W_O projection) and returns a
partial [S, D] output; the host sums the 4 partials per batch.

Schedule, designed around two measured hardware facts:
  - ACT (exp) is the attention-phase floor: ~(N+352)/1.2ns per instruction,
    ~80us total for the causal triangle. ACT does exp and nothing else.
  - PE HAM clock gate: PE idle >~3.4us -> re-throttles to 1.2GHz. PE must
    stay continuously fed (total PE work ~98us @2.4GHz is the kernel floor).

Per-core dataflow:
  - Q^T/K^T in [Dh, S] layout; scores come out transposed so softmax'd P
    needs no transpose for P@V. Softmax denominator via ones-column in V
    (65-wide PV matmuls). Causal mask = multiplicative upper-tri on the
    exp'd bf16 tiles, on DVE (2x rate for 16-bit operands).
  - All matmuls bf16 (fp8 was measured over the error budget: the score
    dot-products random-walk, so element errors don't average out).
    Scores run both head-parities concurrently on PE row-halves (64-deep
    contraction auto row-tiling).
  - Pairs run SEQUENTIALLY within a chunk, software-pipelined ACROSS pair
    boundaries: the next pair's first two scores/exp tiles are emitted
    before the previous pair's PV drain + normalize, so the exp stream
    never gaps. The attention accumulator is 2 rotating 1-bank PSUM tiles,
    leaving a dedicated 2-bank filler pool: QKV-projection and W_O units
    interleave into the kt loop by cycle credit, never waiting on an
    exp-gated scores slot.
  - PSUM: scores 2x[128,1024] (4 banks, exp-paced) + pa 2x[65,512] +
    filler 2x[128,512].
  - All PSUM drains go through DVE. Normalize per (pair,par): DVE row copy
    -> GpSimd partition_broadcast (its ONLY op - mixing ops thrashes its
    ucode library, ~6us per reload) -> DVE reciprocal_approx_fast -> DVE
    mul writing bf16 at tiles (bf16 enables fast weight load for the W_O
    matmuls). The final pair instead broadcasts via a PE ones-matmul and
    splits its drain copies across ACT+DVE: both engines are idle in the
    tail, which puts the last matmul in the same microsecond as the last
    exp.
  - Chunk 3 is ACT-bound (exp ~34us vs ~19us of attention PE work), so the
    V(3) projection units and all wo(2) units are deferred into its kt
    loop: real PE work replaces idle time and keeps the HAM clock warm.
  - Prologue DMAs issue on both HW DGE queues (~190GB/s each),
    critical-first; x quarter 0 lands per-dc so the first QK unit's
    matmuls trickle in behind the DMA.
"""

import numpy as np

try:
    import concourse  # noqa: F401
except ImportError:  # pragma: no cover - harness containers stage it here
    import sys

    sys.path.insert(0, "/opt/trn_rl_repo")

B, S, D, H, DH = 2, 2048, 1024, 16, 64
NCORES = 8
HPC = 4  # heads per core
NPAIR = 2  # head pairs per core
SC = 512  # q-chunk width
NQC = S // SC  # 4 q-chunks
NDC = D // 128  # 8 contraction chunks of 128
VO_W = 65  # V columns + ones column
VO_QSTRIDE = 4 * VO_W  # per-head stride inside one quarter's V|ones tile
LAG = 4  # PV flush lag (tiles) behind exp

_cache = {}


def _build_program():
    from contextlib import ExitStack

    import concourse.mybir as mybir
    import concourse.tile as tile
    from concourse import bacc

    f32 = mybir.dt.float32
    f32r = mybir.dt.float32r
    bf16 = mybir.dt.bfloat16
    AF = mybir.ActivationFunctionType

    nc = bacc.Bacc(
        "TRN2", debug=False, target_bir_lowering=False, num_devices=NCORES
    )

    xT = nc.dram_tensor("xT", [128, NQC * NDC * SC], bf16, kind="ExternalInput").ap()
    wqk = nc.dram_tensor(
        "wqk", [128, 4 * NDC * 128], bf16, kind="ExternalInput"
    ).ap()
    wv = nc.dram_tensor("wv", [128, NDC * 256], bf16, kind="ExternalInput").ap()
    wo = nc.dram_tensor("wo", [128, NPAIR * D], bf16, kind="ExternalInput").ap()
    tri = nc.dram_tensor("tri", [128, 128], bf16, kind="ExternalInput").ap()
    out = nc.dram_tensor("out", [S, D], bf16, kind="ExternalOutput").ap()

    with tile.TileContext(nc) as tc, ExitStack() as ctx:
        persist = ctx.enter_context(tc.tile_pool(name="persist", bufs=1))
        pt_pool = ctx.enter_context(tc.tile_pool(name="pt", bufs=12))
        dens_pool = ctx.enter_context(tc.tile_pool(name="dens", bufs=2))
        denb_pool = ctx.enter_context(tc.tile_pool(name="denb", bufs=2))
        denr_pool = ctx.enter_context(tc.tile_pool(name="denr", bufs=2))
        out_pool = ctx.enter_context(tc.tile_pool(name="outsb", bufs=3))
        sc_pool = ctx.enter_context(tc.tile_pool(name="sc", bufs=2, space="PSUM"))
        fl_pool = ctx.enter_context(tc.tile_pool(name="fl", bufs=2, space="PSUM"))
        pa_pool = ctx.enter_context(tc.tile_pool(name="pa", bufs=2, space="PSUM"))

        # ---- persistent SBUF tensors ----
        x_sb = {
            q: persist.tile([128, NDC * SC], bf16, tag=f"x{q}", name=f"x{q}")
            for q in range(NQC)
        }

        def x_slice(q, dc, lo=0, hi=SC):
            return x_sb[q][:, dc * SC + lo : dc * SC + hi]

        wqk_sb = persist.tile([128, 4 * NDC * 128], bf16, tag="wqk", name="wqk_sb")
        wv_sb = persist.tile([128, NDC * 256], bf16, tag="wv", name="wv_sb")
        wo_sb = persist.tile([128, NPAIR * D], bf16, tag="wo", name="wo_sb")
        trib_sb = persist.tile([128, 128], bf16, tag="trib", name="trib_sb")
        ones_sb = persist.tile([128, 1], f32, tag="ones", name="ones_sb")
        onesr_sb = persist.tile([1, 64], f32r, tag="onesr", name="onesr_sb")
        qt_sb = {
            (p, q): persist.tile([128, SC], bf16, tag=f"qt{p}_{q}", name=f"qt{p}_{q}")
            for p in range(NPAIR)
            for q in range(NQC)
        }
        kt_sb = {
            (p, q): persist.tile([128, SC], bf16, tag=f"kt{p}_{q}", name=f"kt{p}_{q}")
            for p in range(NPAIR)
            for q in range(NQC)
        }
        vo_sb = {
            q: persist.tile(
                [128, HPC * VO_QSTRIDE], bf16, tag=f"vo{q}", name=f"vo{q}"
            )
            for q in range(NQC)
        }
        at_sb = {
            (p, qc): persist.tile(
                [128, SC], bf16, tag=f"at{p}_{qc}", name=f"at{p}_{qc}"
            )
            for p in range(NPAIR)
            for qc in range(NQC)
        }

        # ---- loads: critical-first on two HW DGE queues (~190GB/s each) ----
        BW = NDC * 128  # one (p, qk) weight block
        nc.sync.dma_start(wqk_sb[:, 0:BW], wqk[:, 0:BW])
        for dc in range(NDC):
            nc.sync.dma_start(
                x_sb[0][:, dc * SC : (dc + 1) * SC],
                xT[:, dc * SC : (dc + 1) * SC],
            )
        for q in range(1, NQC):
            nc.sync.dma_start(x_sb[q][:], xT[:, q * NDC * SC : (q + 1) * NDC * SC])
        # scalar queue (ACT idle in prologue): remaining weights. Splitting
        # the x quarter-0 load across both queues was measured SLOWER twice
        # (the scalar HW DGE queue starts later and delays the weight
        # blocks); keep all x on sync
        nc.scalar.dma_start(wqk_sb[:, BW : 2 * BW], wqk[:, BW : 2 * BW])
        nc.scalar.dma_start(wqk_sb[:, 2 * BW : 4 * BW], wqk[:, 2 * BW : 4 * BW])
        nc.scalar.dma_start(wv_sb[:], wv[:])
        nc.scalar.dma_start(trib_sb[:], tri[:])
        nc.scalar.dma_start(wo_sb[:], wo[:])
        nc.vector.memset(ones_sb[:], 1.0)
        nc.vector.tensor_copy(
            onesr_sb[:], ones_sb[0:1, :].to_broadcast((1, 64))
        )
        for q in range(NQC):
            ones_cols = vo_sb[q].rearrange(
                "p (h s w) -> p h s w", h=HPC, w=VO_W
            )[:, :, :, 64]
            nc.vector.tensor_copy(
                ones_cols, ones_sb[:].to_broadcast((128, HPC, 4))
            )

        # ---- unit emitters (each = one filler quantum) ----
        def emit_qk_unit(q, p, qk):
            dst = qt_sb[(p, q)] if qk == 0 else kt_sb[(p, q)]
            ps = fl_pool.tile([128, SC], f32, tag="fl", name=f"psqk{p}{qk}{q}")
            for dc in range(NDC):
                col = ((p * 2 + qk) * NDC + dc) * 128
                nc.tensor.matmul(
                    ps[:],
                    lhsT=wqk_sb[:, col : col + 128],
                    rhs=x_slice(q, dc),
                    start=(dc == 0),
                    stop=(dc == NDC - 1),
                )
            nc.vector.tensor_copy(dst[:], ps[:])

        def emit_v_unit(q, st4):
            ps = fl_pool.tile([128, 256], f32, tag="fl", name=f"psv{q}{st4}")
            for dc in range(NDC):
                nc.tensor.matmul(
                    ps[:],
                    lhsT=x_slice(q, dc, st4 * 128, (st4 + 1) * 128),
                    rhs=wv_sb[:, dc * 256 : (dc + 1) * 256],
                    start=(dc == 0),
                    stop=(dc == NDC - 1),
                )
            vo_cols = vo_sb[q].rearrange(
                "p (h s w) -> p h s w", h=HPC, w=VO_W
            )[:, :, st4, 0:64]
            nc.vector.tensor_copy(
                vo_cols, ps[:].rearrange("p (h e) -> p h e", e=64)
            )

        outt_tiles = {}

        def emit_wo_unit(qc, qt, dch):
            po = fl_pool.tile([128, SC], f32, tag="fl", name=f"po{qc}{qt}{dch}")
            for p in range(NPAIR):
                nc.tensor.matmul(
                    po[:],
                    lhsT=at_sb[(p, qc)][:, qt * 128 : (qt + 1) * 128],
                    rhs=wo_sb[:, p * D + dch * SC : p * D + (dch + 1) * SC],
                    start=(p == 0),
                    stop=(p == NPAIR - 1),
                )
            if dch == 0:
                outt_tiles[(qc, qt)] = out_pool.tile(
                    [128, D], bf16, tag="outsb", name=f"o{qc}{qt}"
                )
            outt = outt_tiles[(qc, qt)]
            if qc == NQC - 1 and (qt + dch) % 2:
                # tail only: ACT is idle once the exp stream ends - split
                # the drain copies across both engines
                nc.scalar.copy(outt[:, dch * SC : (dch + 1) * SC], po[:])
            else:
                nc.vector.tensor_copy(outt[:, dch * SC : (dch + 1) * SC], po[:])
            if dch == 1:
                row = (qc * 4 + qt) * 128
                # full 2KB rows -> one contiguous 256KB DMA. scalar-queue
                # issue costs ~0.7us of ACT time: only borrow it for the
                # last chunk's drain, when the exp stream is done
                last = qc == NQC - 1
                eng = nc.scalar if (last and qt % 2) else nc.sync
                eng.dma_start(out[row : row + 128, :], outt[:])

        def qk_units(q, p):
            return [(4096, "qkv", None,
                     lambda qk=qk: emit_qk_unit(q, p, qk)) for qk in range(2)]

        def v_units(q):
            return [(2048, "qkv", None, lambda st4=st4: emit_v_unit(q, st4))
                    for st4 in range(4)]

        def wo_units(qc):
            return [
                (1024, "wo", qc, lambda qt=qt, dch=dch: emit_wo_unit(qc, qt, dch))
                for qt in range(4)
                for dch in range(2)
            ]

        # ---- attention: one global stream over (qc, pair, kt) with
        # cross-pair software pipelining and cycle-credit fillers ----
        norms_done = {q: 0 for q in range(NQC)}

        def make_closeout(qc, p, pending, pa_par, flush):
            def closeout():
                for pend in pending:
                    flush(*pend)
                dens = {}
                last_pair = qc == NQC - 1 and p == NPAIR - 1
                for par in range(2):
                    dens[par] = dens_pool.tile(
                        [1, SC], f32r if last_pair else f32,
                        tag="dens", name=f"dens{qc}{p}{par}"
                    )
                    nc.vector.tensor_copy(dens[par][:], pa_par[par][64:65, :])

                def norm_fin():
                    for par in range(2):
                        if qc == NQC - 1 and p == NPAIR - 1:
                            # tail: PE is idle and ~2us faster than the
                            # GpSimd broadcast chain here
                            denb = fl_pool.tile(
                                [64, SC], f32, tag="fl", name=f"denb{qc}{p}{par}"
                            )
                            nc.tensor.matmul(
                                denb[:], lhsT=onesr_sb[:], rhs=dens[par][:],
                                start=True, stop=True,
                            )
                        else:
                            denb = denb_pool.tile(
                                [64, SC], f32, tag="denb", name=f"denb{qc}{p}{par}"
                            )
                            nc.gpsimd.partition_broadcast(denb[:], dens[par][:])
                        denr = denr_pool.tile(
                            [64, SC], f32, tag="denr", name=f"denr{qc}{p}{par}"
                        )
                        nc.vector.reciprocal_approx_fast(denr[:], denb[:])
                        nc.vector.tensor_mul(
                            at_sb[(p, qc)][par * 64 : (par + 1) * 64, :],
                            pa_par[par][0:64, :],
                            denr[:],
                        )
                    norms_done[qc] += 1

                return norm_fin

            return closeout

        prev_close = [None]  # closeout thunk of the previous pair
        prev_fin = [None]  # its normalize-finish thunk

        def keep_warm(n):
            # dead-write matmuls that fill dependency stalls in the tail so
            # the HAM clock gate stays at 2.4GHz for the real tail matmuls
            for i in range(n):
                dmy = fl_pool.tile([128, SC], f32, tag="fl", name=f"dmy{i}")
                nc.tensor.matmul(
                    dmy[:], lhsT=wqk_sb[:, 0:128], rhs=x_slice(0, 0),
                    start=True, stop=True,
                )

        def emit_pair(qc, p, filler_state):
            nkt = 4 * (qc + 1)
            pa_par = {}
            pending = []

            def flush(kt, ptile):
                j0 = max(0, kt * 128 - qc * SC)
                kq, kst = kt // 4, kt % 4
                for par in range(2):
                    if kt == 0:
                        pa_par[par] = pa_pool.tile(
                            [VO_W, SC], f32, tag="pa", name=f"pa{qc}{p}{par}"
                        )
                    hh = 2 * p + par
                    vbase = hh * VO_QSTRIDE + kst * VO_W
                    nc.tensor.matmul(
                        pa_par[par][:, j0:SC],
                        lhsT=vo_sb[kq][:, vbase : vbase + VO_W],
                        rhs=ptile.rearrange("p (b n) -> p b n", b=2)[:, par, j0:SC],
                        start=(kt == 0),
                        stop=(kt == nkt - 1),
                    )

            for kt in range(nkt):
                j0 = max(0, kt * 128 - qc * SC)
                kq, kst = kt // 4, kt % 4
                ps_s = sc_pool.tile(
                    [128, 2 * SC], f32, tag="sc", name=f"pss{qc}{p}{kt}"
                )
                for par in range(2):
                    nc.tensor.matmul(
                        ps_s[:, par * SC + j0 : (par + 1) * SC],
                        lhsT=kt_sb[(p, kq)][
                            par * 64 : (par + 1) * 64,
                            kst * 128 : (kst + 1) * 128,
                        ],
                        rhs=qt_sb[(p, qc)][par * 64 : (par + 1) * 64, j0:SC],
                        start=True,
                        stop=True,
                    )
                ptile = pt_pool.tile(
                    [128, 2 * SC], bf16, tag="pt", name=f"pt{qc}{p}{kt}"
                )
                nc.scalar.activation(
                    ptile.rearrange("p (b n) -> p b n", b=2)[:, :, j0:SC],
                    ps_s.rearrange("p (b n) -> p b n", b=2)[:, :, j0:SC],
                    AF.Exp,
                    scale=0.125,
                )
                if kt * 128 >= qc * SC:  # diagonal: multiplicative causal
                    # mask on the exp'd bf16 tile (DVE 2x for 16-bit; GpSimd
                    # must stay partition_broadcast-only or its ucode
                    # library thrashes, ~6us per switch)
                    nc.vector.tensor_mul(
                        ptile.rearrange("p (b n) -> p b n", b=2)[
                            :, :, j0 : j0 + 128
                        ],
                        ptile.rearrange("p (b n) -> p b n", b=2)[
                            :, :, j0 : j0 + 128
                        ],
                        trib_sb[:].unsqueeze(1).to_broadcast((128, 2, 128)),
                    )
                pending.append((kt, ptile))
                final = qc == NQC - 1 and p == NPAIR - 1
                allowed = min(LAG, nkt - kt) if final else LAG
                while len(pending) > allowed:
                    flush(*pending.pop(0))
                # cross-pair pipeline: the previous pair's PV drain + dens
                # pull waits until this pair's kt1 so the exp stream never
                # gaps; its normalize (PE broadcast + DVE) follows at kt2
                if kt == 1 and prev_close[0] is not None:
                    prev_fin[0] = prev_close[0]()
                    prev_close[0] = None
                if kt == 2 and prev_fin[0] is not None:
                    prev_fin[0]()
                    prev_fin[0] = None
                filler_state.step()

            prev_close[0] = make_closeout(qc, p, pending, pa_par, flush)

        class FillerState:
            def __init__(self, units, iters):
                self.units = list(units)
                self.total = sum(u[0] for u in self.units) or 1
                self.done = 0
                self.iters = iters
                self.it = 0

            def step(self):
                self.it += 1
                # normal pacing, but stop 2 iterations before the chunk
                # boundary: nothing may sit between the last scores of this
                # chunk and the first scores of the next in the PE stream
                if self.it > self.iters - 2:
                    return
                target = self.total * self.it // self.iters
                while self.units and self.done < target:
                    picked = None
                    for i, (cyc, kind, dep, thunk) in enumerate(self.units):
                        if kind == "wo" and norms_done[dep] < 2:
                            continue  # at tiles not normalized yet
                        picked = i
                        break
                    if picked is None:
                        break
                    cyc, kind, dep, thunk = self.units.pop(picked)
                    thunk()
                    self.done += cyc

            def remainder(self):
                units, self.units = self.units, []
                return units

        # ---- program ----
        # prologue: just QK(0,p0) so chunk 0's scores/exp start ASAP.
        # (Interleaving Q and K per-dc into both banks was measured ~2us
        # SLOWER: the per-matmul weight alternation defeats the background
        # weight-buffer prefetch.)
        for cyc, kind, dep, thunk in qk_units(0, 0):
            thunk()
        # V(q) units lead chunk q's own filler (needed by its late flushes);
        # V(3) + wo(2) deliberately land in ACT-bound chunk 3, where they
        # replace PE idle time and keep the HAM clock warm
        fillers = {
            0: qk_units(0, 1) + v_units(0) + qk_units(1, 0) + qk_units(1, 1),
            1: v_units(1) + wo_units(0) + qk_units(2, 0) + qk_units(2, 1),
            2: v_units(2) + wo_units(1) + qk_units(3, 0) + qk_units(3, 1),
            3: v_units(3) + wo_units(2),
        }
        carry = []
        for qc in range(NQC):
            fs = FillerState(carry + fillers[qc], NPAIR * 4 * (qc + 1))
            for p in range(NPAIR):
                emit_pair(qc, p, fs)
            carry = fs.remainder()
        prev_fin[0] = prev_close[0]()  # last pair's drain + normalize
        prev_close[0] = None
        keep_warm(4)
        prev_fin[0]()
        keep_warm(2)
        for cyc, kind, dep, thunk in carry + wo_units(NQC - 1):
            if kind == "wo":
                assert norms_done[dep] >= 2
            thunk()

    nc.compile()
    return nc


def _get_program():
    if "nc" not in _cache:
        _cache["nc"] = _build_program()
    return _cache["nc"]


def _prep_core_inputs(c, residual, W_Q, W_K, W_V, W_O, tri):
    import ml_dtypes

    b = c // 4
    heads = [4 * (c % 4) + i for i in range(HPC)]

    def chunked(w):  # [1024, M] -> [128, NDC*M] chunk-major
        m = w.shape[1]
        return np.ascontiguousarray(
            w.reshape(NDC, 128, m).transpose(1, 0, 2).reshape(128, NDC * m)
        )

    wqk_blocks = []
    for p in range(NPAIR):  # block order: Qp0, Kp0, Qp1, Kp1
        for Wt in (W_Q, W_K):
            h0, h1 = heads[2 * p], heads[2 * p + 1]
            wpair = np.concatenate([Wt[h0].T, Wt[h1].T], axis=1)  # [1024, 128]
            wqk_blocks.append(chunked(wpair))
    wqk_arr = np.ascontiguousarray(np.concatenate(wqk_blocks, axis=1))

    wv_arr = chunked(np.concatenate([W_V[h].T for h in heads], axis=1))
    wo_arr = np.ascontiguousarray(
        np.concatenate(
            [
                np.concatenate([W_O[heads[2 * p]], W_O[heads[2 * p + 1]]], axis=0)
                for p in range(NPAIR)
            ],
            axis=1,
        )
    )
    xt = residual[b].T.astype(ml_dtypes.bfloat16)  # [1024, 2048]
    xq = np.concatenate(
        [
            np.concatenate(
                [xt[dc * 128 : (dc + 1) * 128, q * SC : (q + 1) * SC]
                 for dc in range(NDC)], axis=1)
            for q in range(NQC)
        ],
        axis=1,
    )
    return {
        "xT": np.ascontiguousarray(xq),
        "wqk": wqk_arr.astype(ml_dtypes.bfloat16),
        "wv": wv_arr.astype(ml_dtypes.bfloat16),
        "wo": wo_arr.astype(ml_dtypes.bfloat16),
        "tri": tri,
    }


def make_in_maps(residual, W_Q, W_K, W_V, W_O):
    residual = np.asarray(residual, np.float32)
    W_Q, W_K, W_V, W_O = (np.asarray(w, np.float32) for w in (W_Q, W_K, W_V, W_O))
    import ml_dtypes

    # multiplicative causal mask for S^T[k, q] diagonal blocks: keep j >= p
    tri = np.triu(np.ones((128, 128), np.float32)).astype(ml_dtypes.bfloat16)
    return [
        _prep_core_inputs(c, residual, W_Q, W_K, W_V, W_O, tri)
        for c in range(NCORES)
    ]


def gather(results):
    out = np.zeros((B, S, D), np.float64)
    for c in range(NCORES):
        out[c // 4] += results[c]["out"].astype(np.float64)
    return out.astype(np.float32)


def kernel(residual, W_Q, W_K, W_V, W_O, **run_kwargs):
    from concourse.bass_utils import run_bass_kernel_spmd

    nc = _get_program()
    in_maps = make_in_maps(residual, W_Q, W_K, W_V, W_O)
    res = run_bass_kernel_spmd(nc, in_maps, core_ids=list(range(NCORES)), **run_kwargs)
    out = gather(res.results)
    if run_kwargs:
        _cache["last_results"] = res
    return out
